# revision 1
# baseline (speedup 1.0000x reference)
"""Trainium2 Bass kernel for CrossNetGatingMixLayer.

Math (per layer i, with U,C,V per expert e; gate = softmax over a singleton
axis == 1.0 identically, so the gating einsum and G are dead code):

    xv = tanh(xl @ V[e])          (B,R)  per expert
    xc = tanh(xv @ C[e].T)        (B,R)
    xu = xc @ U[e].T              (B,D)
    xl = xl + x0 * (sum_e xu + E * bias)

Strategy: data-parallel over 8 NeuronCores (batch split 16384 -> 8 x 2048).
All device-side work is pure matmul: the host pre-transposes x (fed as
x^T [D, BL]), U (fed as U^T chunks) and C (fed as C^T), so the PE never
runs transpose instructions; the output is written transposed [D, BL] and
un-transposed on the host.  The whole data path is bf16 (matmul PSUM
accumulation stays fp32; measured end-to-end rel err ~4e-3), which halves
every DMA transfer and doubles DVE throughput on the residual adds.
Elementwise cross/residual work is split across the Pool (mul) and DVE
(add) engines; a short burst of warm-up matmuls on zeroed tiles walks the
PE through its p-state ramp while the first DMAs are in flight, and the
last layer's final 512 columns are split 256+256 with their drains spread
across engines so the end-of-kernel chain is short.
"""
import numpy as np
import ml_dtypes
from contextlib import ExitStack

import concourse.bass as bass
from concourse import bacc
import concourse.mybir as mybir
import concourse.tile as tile
from concourse.bass_utils import run_bass_kernel_spmd

B, D, L, E, R = 16384, 512, 3, 4, 128
NCORES = 8
BL = B // NCORES            # 2048 rows per core
NBC = BL // 512             # 4 batch chunks of 512 (matmul free dim)
ND = D // 128               # 4 d-chunks of 128
f32 = mybir.dt.float32
bf16 = mybir.dt.bfloat16
Tanh = mybir.ActivationFunctionType.Tanh

_prog_cache = {}


def _build(has_bias: bool):
    nc = bacc.Bacc("TRN2")
    # Host-prepared layouts (see _run), all bf16:
    #   xT:  x^T per core, [D, BL]
    #   Wv:  V[l,e][kd*128+p, r]          -> [128, L, E, ND, R]
    #   Wc:  C[l,e]^T[s, r]               -> [128, L, E, R]
    #   Wu:  U[l,e]^T[r, dc*128+dd]       -> [128, L, E, ND, 128]
    #   b:   E * bias, [L, D] fp32  (only when nonzero)
    xT_d = nc.declare_dram_parameter("xT", [D, BL], bf16, isOutput=False)
    Wv_d = nc.declare_dram_parameter("Wv", [128, L, E, ND, R], bf16,
                                     isOutput=False)
    Wc_d = nc.declare_dram_parameter("Wc", [128, L, E, R], bf16,
                                     isOutput=False)
    Wu_d = nc.declare_dram_parameter("Wu", [128, L, E, ND, 128], bf16,
                                     isOutput=False)
    if has_bias:
        b_d = nc.declare_dram_parameter("b", [L, D], f32, isOutput=False)
    out_d = nc.declare_dram_parameter("out", [D, BL], bf16, isOutput=True)

    xT_r = xT_d.rearrange("(dc p) b -> p dc b", p=128)
    out_r = out_d.rearrange("(dc p) b -> p dc b", p=128)

    with tile.TileContext(nc) as tc, ExitStack() as ctx:
        wpool = ctx.enter_context(tc.tile_pool(name="wpool", bufs=1))
        xpool = ctx.enter_context(tc.tile_pool(name="xpool", bufs=1))
        hr_p = ctx.enter_context(tc.tile_pool(name="hr_p", bufs=5))
        z_p = ctx.enter_context(tc.tile_pool(name="z_p", bufs=6))
        tmp_p = ctx.enter_context(tc.tile_pool(name="tmp_p", bufs=4))
        ph_p = ctx.enter_context(tc.tile_pool(name="ph_p", bufs=3, space="PSUM"))
        pz_p = ctx.enter_context(tc.tile_pool(name="pz_p", bufs=2, space="PSUM"))
        pu_p = ctx.enter_context(tc.tile_pool(name="pu_p", bufs=3, space="PSUM"))

        xlT = xpool.tile([128, ND, BL], bf16)   # residual stream
        x0 = xpool.tile([128, ND, BL], bf16)    # original x, layers >= 1
        Vb = wpool.tile([128, L, E, ND, R], bf16)
        Cb = wpool.tile([128, L, E, R], bf16)
        Ub = wpool.tile([128, L, E, ND, 128], bf16)
        if has_bias:
            ebias = wpool.tile([128, L, ND], f32)

        # ---- PE p-state warm-up: run zero matmuls while the first real
        # DMAs are still in flight so the ~3us ramp burns on junk work.
        with tc.tile_pool(name="warm", bufs=1) as warm_p:
            jw = warm_p.tile([128, 128], bf16)
            jx = warm_p.tile([128, 512], bf16)
            nc.gpsimd.memset(jw, 0.0)
            nc.vector.memset(jx, 0.0)
            pj = pu_p.tile([128, 512], f32, name="pjwarm", tag="pu")
            for _ in range(6):
                nc.tensor.matmul(pj, lhsT=jw, rhs=jx, start=True, stop=True)

        # ---- DMA schedule: first chunk + layer-0 weights first, the rest
        # ordered by first use.
        def ld_x(dst, c, src_r=None):
            r = xT_r if src_r is None else src_r
            nc.sync.dma_start(out=dst[:, :, 512 * c:512 * (c + 1)],
                              in_=r[:, :, 512 * c:512 * (c + 1)])

        def ld_x_dc(dst, c, dc, src_r=None):
            r = xT_r if src_r is None else src_r
            nc.sync.dma_start(out=dst[:, dc, 512 * c:512 * (c + 1)],
                              in_=r[:, dc, 512 * c:512 * (c + 1)])

        def ld_v(l, e):
            nc.sync.dma_start(out=Vb[:, l, e], in_=Wv_d[:, l, e])

        def ld_c(l):
            nc.sync.dma_start(out=Cb[:, l], in_=Wc_d[:, l])

        def ld_u(l, e=None):
            if e is None:
                nc.sync.dma_start(out=Ub[:, l], in_=Wu_d[:, l])
            else:
                nc.sync.dma_start(out=Ub[:, l, e], in_=Wu_d[:, l, e])

        # V[0,e0] first (lhsT for every kd), then chunk-0 x per d-chunk so
        # the first accumulation group starts as early as possible.
        ld_v(0, 0)
        for dc in range(ND):
            ld_x_dc(xlT, 0, dc)
        ld_v(0, 1)
        ld_c(0)
        ld_v(0, 2)
        ld_v(0, 3)
        for e in range(E):
            ld_u(0, e)
        ld_x(xlT, 1)
        for e in range(E):
            ld_v(1, e)
        ld_c(1)
        ld_u(1)
        ld_x(xlT, 2)
        ld_x(xlT, 3)
        ld_x(x0, 0)
        if has_bias:
            nc.sync.dma_start(
                out=ebias, in_=b_d.rearrange("l (dc p) -> p l dc", p=128))
        ld_x(x0, 1)
        ld_x(x0, 2)
        ld_x(x0, 3)
        for e in range(E):
            ld_v(2, e)
        ld_c(2)
        ld_u(2)

        # ---- main loop: 3 layers x 4 chunks of 512 batch columns.  The
        # matmul chunks stay 512 wide everywhere (the Act engine's fixed
        # per-op PSUM access latency makes narrower chunks tanh-bound); only
        # the very last chunk's drain is subdivided into 256-column pieces
        # spread across engines so the end-of-kernel mul/add/DMA chain is
        # short.
        full = [(512 * c, 512) for c in range(NBC)]
        for l in range(L):
            last = l == L - 1
            # bf16 matmuls run at 1 cycle/row at any free size, so the last
            # layer can taper its final chunk to 128 columns, quartering the
            # exposed end-of-kernel drain (DVE mul -> add -> DMA) chain.
            chunks = full[:-1] + [(1536, 384), (1920, 128)] if last else full
            for ci, (c0, w) in enumerate(chunks):
                cols = slice(c0, c0 + w)
                zs = []
                for e in range(E):
                    ph = ph_p.tile([128, w], f32, name=f"ph{l}_{ci}_{e}",
                                   tag="ph")
                    for kd in range(ND):
                        nc.tensor.matmul(
                            ph,
                            lhsT=Vb[:, l, e, kd, :],
                            rhs=xlT[:, kd, cols],
                            start=(kd == 0), stop=(kd == ND - 1))
                    hr = hr_p.tile([128, w], bf16, name=f"h{l}_{ci}_{e}",
                                   tag="h")
                    nc.scalar.activation(hr, ph, Tanh)

                    pz = pz_p.tile([128, w], f32, name=f"pz{l}_{ci}_{e}",
                                   tag="pz")
                    nc.tensor.matmul(pz, lhsT=Cb[:, l, e, :], rhs=hr,
                                     start=True, stop=True)
                    z = z_p.tile([128, w], bf16, name=f"z{l}_{ci}_{e}",
                                 tag="z")
                    nc.scalar.activation(z, pz, Tanh)
                    zs.append(z)

                final = last and ci == NBC - 1
                for dc in range(ND):
                    pu = pu_p.tile([128, w], f32, name=f"pu{l}_{ci}_{dc}",
                                   tag="pu")
                    for e in range(E):
                        nc.tensor.matmul(
                            pu, lhsT=Ub[:, l, e, dc, :], rhs=zs[e],
                            start=(e == 0), stop=(e == E - 1))
                    # drain: PSUM is only readable by DVE (and Act), so the
                    # cross-multiply runs on DVE with a bf16 output cast; the
                    # residual add is all-bf16 and runs on Pool, except the
                    # final d-chunk of the kernel which takes the fast DVE
                    # 2x-bf16 path to shorten the closing chain.
                    x0c = (xlT if l == 0 else x0)
                    tmp = tmp_p.tile([128, w], bf16,
                                     name=f"tmp{l}_{ci}_{dc}", tag="tmp")
                    if has_bias:
                        nc.vector.scalar_tensor_tensor(
                            tmp, pu, ebias[:, l, dc], x0c[:, dc, cols],
                            mybir.AluOpType.add, mybir.AluOpType.mult)
                    else:
                        nc.vector.tensor_mul(tmp, pu, x0c[:, dc, cols])
                    add_e = nc.vector if (final and dc == ND - 1) \
                        else nc.gpsimd
                    add_e.tensor_add(
                        xlT[:, dc, cols], xlT[:, dc, cols], tmp)
                    if last:
                        dma_e = nc.scalar if (final and dc % 2) else nc.sync
                        dma_e.dma_start(
                            out=out_r[:, dc, cols],
                            in_=xlT[:, dc, cols])

    nc.finalize()
    return nc


def _get_prog(has_bias: bool):
    if has_bias not in _prog_cache:
        _prog_cache[has_bias] = _build(has_bias)
    return _prog_cache[has_bias]


def _prep_inputs(inputs):
    x = np.ascontiguousarray(np.asarray(inputs["x"], dtype=np.float32))
    Us = np.asarray(inputs["Us"], dtype=np.float32)
    Cs = np.asarray(inputs["Cs"], dtype=np.float32)
    Vs = np.asarray(inputs["Vs"], dtype=np.float32)
    b = np.asarray(inputs["b"], dtype=np.float32)
    assert x.shape == (B, D), x.shape

    xT = np.ascontiguousarray(x.T).astype(ml_dtypes.bfloat16)       # [D, B]
    Wv = np.ascontiguousarray(
        Vs.reshape(L, E, ND, 128, R).transpose(3, 0, 1, 2, 4)
    ).astype(ml_dtypes.bfloat16)
    Wc = np.ascontiguousarray(
        Cs.transpose(3, 0, 1, 2)).astype(ml_dtypes.bfloat16)
    Wu = np.ascontiguousarray(
        Us.reshape(L, E, ND, 128, R).transpose(4, 0, 1, 2, 3)
    ).astype(ml_dtypes.bfloat16)
    return xT, Wv, Wc, Wu, b


def _run(inputs, trace=False):
    xT, Wv, Wc, Wu, b = _prep_inputs(inputs)
    has_bias = bool(np.any(b))
    nc = _get_prog(has_bias)
    in_maps = []
    for i in range(NCORES):
        m = {"xT": np.ascontiguousarray(xT[:, i * BL:(i + 1) * BL]),
             "Wv": Wv, "Wc": Wc, "Wu": Wu}
        if has_bias:
            m["b"] = np.ascontiguousarray(b * float(E))
        in_maps.append(m)
    res = run_bass_kernel_spmd(nc, in_maps, core_ids=list(range(NCORES)),
                               trace=trace)
    out = np.concatenate(
        [np.asarray(res.results[i]["out"]).astype(np.float32).T
         for i in range(NCORES)], axis=0)
    return np.ascontiguousarray(out), res


def kernel(**inputs) -> np.ndarray:
    out, _ = _run(inputs)
    return out



# revision 3
# speedup vs baseline: 1.2041x; 1.2041x over previous
"""Trainium2 Bass kernel for CrossNetGatingMixLayer.

Math (per layer i, with U,C,V per expert e; gate = softmax over a singleton
axis == 1.0 identically, so the gating einsum and G are dead code):

    xv = tanh(xl @ V[e])          (B,R)  per expert
    xc = tanh(xv @ C[e].T)        (B,R)
    xu = xc @ U[e].T              (B,D)
    xl = xl + x0 * (sum_e xu + E * bias)

Strategy: data-parallel over 8 NeuronCores (batch split 16384 -> 8 x 2048).
The big D-dimension matmuls (V and U) run in fp8-e4m3 DoubleRow mode, which
the PE executes at 0.5 cycles/row (2 k-tiles of 128 per instruction), i.e.
4x the bf16 rate.  Accuracy is recovered with *split-fp8* operands:

  - every fp8 weight is host-split into hi = e4m3(16*W) and a same-scale
    residual lo = e4m3(16*W - hi), so weight precision is ~bf16-class while
    both halves stream through fp8 DoubleRow k-tile pairs;
  - the V matmul's activation xl is likewise split (hi + lo words, the lo
    produced on-device by one subtract per tile), giving three DoubleRow
    passes hi*hi + hi*lo + lo*hi per expert;
  - the C matmul (R x R, cheap) stays bf16 on the bf16 tanh output, so the
    only single-fp8 tensor in the chain is z = tanh2's fp8 output feeding U.

Measured (numpy bit-model, validated against hw on the bf16 baseline):
rel err ~1.8e-2 vs the 2e-2 gate.  Engine budget per core: PE ~61us,
Act (tanh, 1024-wide ops) ~50us, DVE (PSUM drains + residual adds + lo
subs) ~48us, Pool (fp8 hi casts) ~23us.  The U stage is software-pipelined
one (layer, chunk) iteration behind V/C so the PE never waits on tanh2.
"""
import numpy as np
import ml_dtypes
from contextlib import ExitStack

import concourse.bass as bass
from concourse import bacc
import concourse.mybir as mybir
import concourse.tile as tile
from concourse.bass_utils import run_bass_kernel_spmd

B, D, L, E, R = 16384, 512, 3, 4, 128
NCORES = 8
BL = B // NCORES            # 2048 rows per core
NBC = BL // 512             # 4 batch chunks of 512 (matmul free dim)
ND = D // 128               # 4 d-chunks of 128
f32 = mybir.dt.float32
bf16 = mybir.dt.bfloat16
f8 = mybir.dt.float8e4
Tanh = mybir.ActivationFunctionType.Tanh
DR = mybir.MatmulPerfMode.DoubleRow
np_f8 = ml_dtypes.float8_e4m3
np_bf16 = ml_dtypes.bfloat16
SW = 16.0                   # fp8 weight scale (V, U)

_prog_cache = {}


def _build(has_bias: bool):
    nc = bacc.Bacc("TRN2")
    # Host-prepared layouts (see _prep_inputs):
    #   x8h/x8l: split-fp8 x^T per core, [D, BL] (overwritten in place with
    #            the split of the evolving residual for layers 1, 2)
    #   xT:      x^T bf16 [D, BL]  (residual stream)
    #   x0s:     x^T / 16 bf16 [D, BL]  (cross-multiply operand, pre-unscaled)
    #   Vh/Vl:   split e4m3(16*V)   [128(dk), L, E, ND, R]
    #   Cb:      C^T bf16           [128(s),  L, E, R]
    #   Uh/Ul:   split e4m3(16*U)   [128(r),  L, E, ND, 128(dd)]
    #   eb:      16 * E * bias, [L, D] f32  (only when nonzero)
    x8h_d = nc.declare_dram_parameter("x8h", [D, BL], f8, isOutput=False)
    x8l_d = nc.declare_dram_parameter("x8l", [D, BL], f8, isOutput=False)
    xT_d = nc.declare_dram_parameter("xT", [D, BL], bf16, isOutput=False)
    x0s_d = nc.declare_dram_parameter("x0s", [D, BL], bf16, isOutput=False)
    Vh_d = nc.declare_dram_parameter("Vh", [128, L, E, ND, R], f8,
                                     isOutput=False)
    Vl_d = nc.declare_dram_parameter("Vl", [128, L, E, ND, R], f8,
                                     isOutput=False)
    Cb_d = nc.declare_dram_parameter("Cb", [128, L, E, R], bf16,
                                     isOutput=False)
    Uh_d = nc.declare_dram_parameter("Uh", [128, L, E, ND, 128], f8,
                                     isOutput=False)
    Ul_d = nc.declare_dram_parameter("Ul", [128, L, E, ND, 128], f8,
                                     isOutput=False)
    if has_bias:
        eb_d = nc.declare_dram_parameter("eb", [L, D], f32, isOutput=False)
    out_d = nc.declare_dram_parameter("out", [D, BL], bf16, isOutput=True)

    x8h_r = x8h_d.rearrange("(dc p) b -> p dc b", p=128)
    x8l_r = x8l_d.rearrange("(dc p) b -> p dc b", p=128)
    xT_r = xT_d.rearrange("(dc p) b -> p dc b", p=128)
    x0s_r = x0s_d.rearrange("(dc p) b -> p dc b", p=128)
    out_r = out_d.rearrange("(dc p) b -> p dc b", p=128)

    with tile.TileContext(nc) as tc, ExitStack() as ctx:
        wpool = ctx.enter_context(tc.tile_pool(name="wpool", bufs=1))
        xpool = ctx.enter_context(tc.tile_pool(name="xpool", bufs=1))
        hr_p = ctx.enter_context(tc.tile_pool(name="hr_p", bufs=3))
        z_p = ctx.enter_context(tc.tile_pool(name="z_p", bufs=5))
        tmp_p = ctx.enter_context(tc.tile_pool(name="tmp_p", bufs=4))
        # PSUM: ph and pu share one pool of 2-bank tiles (4 banks), pz gets
        # its own 2-bank x2 (4 banks) -> exactly 8 banks.
        phu_p = ctx.enter_context(tc.tile_pool(name="phu_p", bufs=2,
                                               space="PSUM"))
        pz_p = ctx.enter_context(tc.tile_pool(name="pz_p", bufs=2,
                                              space="PSUM"))

        xlT = xpool.tile([128, ND, BL], bf16)     # residual stream
        x0s = xpool.tile([128, ND, BL], bf16)
        x8h = xpool.tile([128, ND, BL], f8)       # split-fp8 of current xl
        x8l = xpool.tile([128, ND, BL], f8)
        Vh = wpool.tile([128, L, E, ND, R], f8)
        Vl = wpool.tile([128, L, E, ND, R], f8)
        Cb = wpool.tile([128, L, E, R], bf16)
        Uh = wpool.tile([128, L, E, ND, 128], f8)
        Ul = wpool.tile([128, L, E, ND, 128], f8)
        if has_bias:
            ebias = wpool.tile([128, L, ND], f32)

        # ---- PE p-state warm-up: zero matmuls while the first DMAs fly.
        with tc.tile_pool(name="warm", bufs=1) as warm_p:
            jw = warm_p.tile([128, 128], bf16)
            jx = warm_p.tile([128, 512], bf16)
            nc.gpsimd.memset(jw, 0.0)
            nc.vector.memset(jx, 0.0)
            pj = pz_p.tile([128, 2, 512], f32, name="pjwarm", tag="pz")
            for _ in range(8):
                nc.tensor.matmul(pj[:, 0, :], lhsT=jw, rhs=jx,
                                 start=True, stop=True)

        # ---- DMA schedule, ordered by first use (all on the sync queue).
        def ld_x(dst, src_r, c):
            nc.sync.dma_start(out=dst[:, :, 512 * c:512 * (c + 1)],
                              in_=src_r[:, :, 512 * c:512 * (c + 1)])

        nc.sync.dma_start(out=Vh[:, 0], in_=Vh_d[:, 0])
        ld_x(x8h, x8h_r, 0)
        nc.sync.dma_start(out=Vl[:, 0], in_=Vl_d[:, 0])
        ld_x(x8l, x8l_r, 0)
        nc.sync.dma_start(out=Cb[:, 0], in_=Cb_d[:, 0])
        nc.sync.dma_start(out=Uh[:, 0], in_=Uh_d[:, 0])
        nc.sync.dma_start(out=Ul[:, 0], in_=Ul_d[:, 0])
        ld_x(x8h, x8h_r, 1)
        ld_x(x8l, x8l_r, 1)
        ld_x(xlT, xT_r, 0)
        ld_x(x0s, x0s_r, 0)
        if has_bias:
            nc.sync.dma_start(
                out=ebias, in_=eb_d.rearrange("l (dc p) -> p l dc", p=128))
        ld_x(x8h, x8h_r, 2)
        ld_x(x8l, x8l_r, 2)
        ld_x(xlT, xT_r, 1)
        ld_x(x0s, x0s_r, 1)
        nc.sync.dma_start(out=Vh[:, 1], in_=Vh_d[:, 1])
        nc.sync.dma_start(out=Vl[:, 1], in_=Vl_d[:, 1])
        nc.sync.dma_start(out=Cb[:, 1], in_=Cb_d[:, 1])
        nc.sync.dma_start(out=Uh[:, 1], in_=Uh_d[:, 1])
        nc.sync.dma_start(out=Ul[:, 1], in_=Ul_d[:, 1])
        ld_x(x8h, x8h_r, 3)
        ld_x(x8l, x8l_r, 3)
        ld_x(xlT, xT_r, 2)
        ld_x(x0s, x0s_r, 2)
        ld_x(xlT, xT_r, 3)
        ld_x(x0s, x0s_r, 3)
        nc.sync.dma_start(out=Vh[:, 2], in_=Vh_d[:, 2])
        nc.sync.dma_start(out=Vl[:, 2], in_=Vl_d[:, 2])
        nc.sync.dma_start(out=Cb[:, 2], in_=Cb_d[:, 2])
        nc.sync.dma_start(out=Uh[:, 2], in_=Uh_d[:, 2])
        nc.sync.dma_start(out=Ul[:, 2], in_=Ul_d[:, 2])

        # ---- main loop.  12 iterations of (layer, 512-col chunk).  Per
        # iteration: V DoubleRow passes + tanh1 + bf16 C + tanh2 for both
        # expert pairs, then the *previous* iteration's U matmuls + drains
        # (software-pipelined so the PE never waits on this iteration's
        # tanh2).
        def emit_U(st):
            l, ci, zA, zB = st
            cols = slice(512 * ci, 512 * (ci + 1))
            last = l == L - 1
            for half in range(2):          # dc pairs (0,1) and (2,3)
                dcp = slice(2 * half, 2 * half + 2)
                pu = phu_p.tile([128, 2, 512], f32,
                                name=f"pu{l}_{ci}_{half}", tag="phu")
                for j in range(2):         # dc within pair
                    dc = 2 * half + j
                    for k, (Ut, zt) in enumerate(
                            ((Uh, zA), (Uh, zB), (Ul, zA), (Ul, zB))):
                        ep = slice(0, 2) if (k % 2 == 0) else slice(2, 4)
                        nc.tensor.matmul(
                            pu[:, j, :], lhsT=Ut[:, l, ep, dc, :], rhs=zt,
                            start=(k == 0), stop=(k == 3), perf_mode=DR)
                tmp = tmp_p.tile([128, 2, 512], bf16,
                                 name=f"tmp{l}_{ci}_{half}", tag="tmp")
                if has_bias:
                    for j in range(2):
                        nc.vector.scalar_tensor_tensor(
                            tmp[:, j, :], pu[:, j, :],
                            ebias[:, l, 2 * half + j],
                            x0s[:, 2 * half + j, cols],
                            mybir.AluOpType.add, mybir.AluOpType.mult)
                else:
                    nc.vector.tensor_mul(tmp, pu, x0s[:, dcp, cols])
                nc.vector.tensor_add(
                    xlT[:, dcp, cols], xlT[:, dcp, cols], tmp)
                if not last:
                    # split-fp8 of the updated residual for the next layer
                    nc.gpsimd.tensor_copy(x8h[:, dcp, cols],
                                          xlT[:, dcp, cols])
                    nc.vector.tensor_sub(x8l[:, dcp, cols],
                                         xlT[:, dcp, cols],
                                         x8h[:, dcp, cols])
                else:
                    nc.sync.dma_start(out=out_r[:, dcp, cols],
                                      in_=xlT[:, dcp, cols])

        pend = None
        for l in range(L):
            for ci in range(NBC):
                cols = slice(512 * ci, 512 * (ci + 1))
                phs = []
                for p in range(2):         # expert pairs
                    ph = phu_p.tile([128, 2, 512], f32,
                                    name=f"ph{l}_{ci}_{p}", tag="phu")
                    for j in range(2):
                        e = 2 * p + j
                        passes = ((Vh, x8h), (Vh, x8l), (Vl, x8h))
                        n = 0
                        for Wt, xt in passes:
                            for kp in range(2):
                                nc.tensor.matmul(
                                    ph[:, j, :],
                                    lhsT=Wt[:, l, e, 2 * kp:2 * kp + 2, :],
                                    rhs=xt[:, 2 * kp:2 * kp + 2, cols],
                                    start=(n == 0), stop=(n == 5),
                                    perf_mode=DR)
                                n += 1
                    phs.append(ph)
                zs = []
                for p in range(2):
                    hr = hr_p.tile([128, 2, 512], bf16,
                                   name=f"h{l}_{ci}_{p}", tag="h")
                    nc.scalar.activation(hr, phs[p], Tanh, scale=1.0 / SW)
                    pz = pz_p.tile([128, 2, 512], f32,
                                   name=f"pz{l}_{ci}_{p}", tag="pz")
                    for j in range(2):
                        nc.tensor.matmul(pz[:, j, :],
                                         lhsT=Cb[:, l, 2 * p + j, :],
                                         rhs=hr[:, j, :],
                                         start=True, stop=True)
                    z8 = z_p.tile([128, 2, 512], f8,
                                  name=f"z{l}_{ci}_{p}", tag="z")
                    nc.scalar.activation(z8, pz, Tanh)
                    zs.append(z8)
                if pend is not None:
                    emit_U(pend)
                pend = (l, ci, zs[0], zs[1])
        emit_U(pend)

    nc.finalize()
    return nc


def _get_prog(has_bias: bool):
    if has_bias not in _prog_cache:
        _prog_cache[has_bias] = _build(has_bias)
    return _prog_cache[has_bias]


def _split8(a):
    hi = a.astype(np_f8)
    lo = (a - hi.astype(np.float32)).astype(np_f8)
    return hi, lo


def _prep_inputs(inputs):
    x = np.asarray(inputs["x"], dtype=np.float32)
    Us = np.asarray(inputs["Us"], dtype=np.float32)
    Cs = np.asarray(inputs["Cs"], dtype=np.float32)
    Vs = np.asarray(inputs["Vs"], dtype=np.float32)
    b = np.asarray(inputs["b"], dtype=np.float32)
    assert x.shape == (B, D), x.shape

    xTb = np.ascontiguousarray(x.T).astype(np_bf16)          # [D, B] bf16
    x8h, x8l = _split8(xTb.astype(np.float32))
    x0s = (np.ascontiguousarray(x.T) / SW).astype(np_bf16)

    Vh, Vl = _split8(SW * Vs.reshape(L, E, ND, 128, R).transpose(3, 0, 1, 2, 4))
    Uh, Ul = _split8(SW * Us.reshape(L, E, ND, 128, R).transpose(4, 0, 1, 2, 3))
    Cb = np.ascontiguousarray(Cs.transpose(3, 0, 1, 2)).astype(np_bf16)
    full = {"xT": xTb, "x8h": np.ascontiguousarray(x8h),
            "x8l": np.ascontiguousarray(x8l), "x0s": x0s,
            "Vh": np.ascontiguousarray(Vh), "Vl": np.ascontiguousarray(Vl),
            "Cb": Cb, "Uh": np.ascontiguousarray(Uh),
            "Ul": np.ascontiguousarray(Ul)}
    if np.any(b):
        full["eb"] = np.ascontiguousarray(SW * E * b)
    return full


def _core_inputs(full, i):
    cols = slice(i * BL, (i + 1) * BL)
    m = {}
    for k, v in full.items():
        if k in ("xT", "x8h", "x8l", "x0s"):
            m[k] = np.ascontiguousarray(v[:, cols])
        else:
            m[k] = v
    return m


def _run(inputs, trace=False):
    full = _prep_inputs(inputs)
    has_bias = "eb" in full
    nc = _get_prog(has_bias)
    in_maps = [_core_inputs(full, i) for i in range(NCORES)]
    res = run_bass_kernel_spmd(nc, in_maps, core_ids=list(range(NCORES)),
                               trace=trace)
    out = np.concatenate(
        [np.asarray(res.results[i]["out"]).astype(np.float32).T
         for i in range(NCORES)], axis=0)
    return np.ascontiguousarray(out), res


def kernel(**inputs) -> np.ndarray:
    out, _ = _run(inputs)
    return out
